# revision 25
# baseline (speedup 1.0000x reference)
"""BNext block (attention + FFN_1x1, binarized convs, frozen BN) on 8 TRN2 cores.

Data-parallel over batch (16 -> 2 images per core). Per core:
  - channels on partitions (2 c-tiles), pixels (b, h, w) on the free dim
  - binary activations: the ct0 half uses +-1 encoding written by ScalarE
    Sign; the ct1 half uses {0,1} encoding written by DVE is_ge (z = 2b-1,
    conv borders memset to 0.5 so padding taps contribute zero). The
    -sum(w) correction for the {0,1} half is folded into the bn1 bias
    host-side; per-half weight scales (512 / 1024 for fp8 range) likewise.
  - 3x3 conv runs as fp8 DoubleRow matmuls (K=256 packed as [128, 2, .]
    pairs, 0.5 cyc/row); the 1x1 ffn runs as bf16 matmuls so the DVE z2
    half hits the 2x mode; bn+prelu fused into the PSUM drains via ScalarE
    Prelu with accum_out producing the SE pooling sums. bn2's inv2/finv2
    are folded into the drains (prelu is positive-homogeneous), and the
    SE-pool bias corrections into the SE hidden bias, so the gates are a
    bare sigmoid.
  - engine balance: ScalarE = ct0 signs + residual prelu + drains; DVE =
    ct1 signs + pooling reduce + mix/residual/final algebra (all bf16 2x);
    GPSIMD does nothing big (its sw elementwise is 5-20x slower than DVE);
    SP issues x/out DMA, ACT SEQ issues the merged constant DMAs behind
    the x stream.
  - x streamed in as bf16, output written as bf16 (host upcasts)
  - per-image pipelining: image 0's SE gate + postprocessing overlap image
    1's conv matmuls
"""

import numpy as np

EPS = 1e-5
NCORES = 8
B, C, H, W = 16, 256, 56, 56
BP = B // NCORES            # images per core
HW = H * W                  # 3136
PIX = BP * HW               # 6272
CT = C // 128               # 2 c-tiles
HP, WPP = H + 2, 64         # padded z image: 58 rows x 64 cols (56+2 used)
RS = 8                      # conv chunk rows  -> N = 448
NCH = H // RS               # 7 conv chunks per image
SR2 = 28                    # A1/A4/F3 chunk rows (1568 elems)
NSC2 = H // SR2             # 2 per image
NV = 20

_CACHE = {}


def _build_program(loop_R=None, phase_limit=99, bench_mode=False):
    import concourse.bass as bass
    import concourse.bacc as bacc
    import concourse.tile as tile
    from concourse import mybir

    AF = mybir.ActivationFunctionType
    ALU = mybir.AluOpType
    F32 = mybir.dt.float32
    BF16 = mybir.dt.bfloat16
    FP8 = mybir.dt.float8e4
    DR = mybir.MatmulPerfMode.DoubleRow

    nc = bacc.Bacc("TRN2", target_bir_lowering=False, debug=False)

    KIN = "Internal" if bench_mode else "ExternalInput"
    KOUT = "Internal" if bench_mode else "ExternalOutput"
    xin = nc.dram_tensor("xin", [BP, C, H, W], BF16, kind=KIN).ap()
    # weight DRAM layouts are partition-first so each loads in ONE dma
    wa = nc.dram_tensor("wa", [128, 9, CT, 2, 128], FP8, kind=KIN).ap()
    wf = nc.dram_tensor("wf", [128, CT, CT, 128], BF16, kind=KIN).ap()
    vecs_d = nc.dram_tensor("vecs", [128, CT, NV], F32, kind=KIN).ap()
    b1_d = nc.dram_tensor("b1", [32, 2], F32, kind=KIN).ap()
    # host folds s / (1-s), 1/HW and 1/inv2 into the first SE matmul weights
    w1a_d = nc.dram_tensor("w1a", [128, CT, 2, 32], F32, kind=KIN).ap()
    w2a_d = nc.dram_tensor("w2a", [32, C], F32, kind=KIN).ap()
    w1f_d = nc.dram_tensor("w1f", [128, CT, 2, 32], F32, kind=KIN).ap()
    w2f_d = nc.dram_tensor("w2f", [32, C], F32, kind=KIN).ap()
    out_d = nc.dram_tensor("out", [BP, C, H, W], BF16, kind=KOUT).ap()
    tick_d = (nc.dram_tensor("tick", [1, 8], F32, kind="ExternalOutput").ap()
              if bench_mode else None)

    x_v = xin.rearrange("b (ct p) h w -> ct p b (h w)", ct=CT)
    out_v = out_d.rearrange("b (ct p) h w -> ct p b (h w)", ct=CT)

    (V_MOVE, V_AL1, V_INV1, V_BIAS1, V_AL2, V_NMOVE, V_1MS, V_ZB2, V_NZB2,
     V_FINV1, V_FBIAS1, V_FAL2, V_FS, V_1MFS, V_FINV2, V_CFIN, V_B2A,
     V_B2F, V_SB2HW, V_UNUSED) = range(NV)

    with tile.TileContext(nc) as tc:
        import contextlib
        es = contextlib.ExitStack()
        with es:
            consts = es.enter_context(tc.tile_pool(name="consts", bufs=1))
            big = es.enter_context(tc.tile_pool(name="big", bufs=1))
            stream = es.enter_context(tc.tile_pool(name="stream", bufs=2))
            psum_c = es.enter_context(
                tc.tile_pool(name="psum_c", bufs=5, space="PSUM"))
            psum_f = es.enter_context(
                tc.tile_pool(name="psum_f", bufs=2, space="PSUM"))
            psum_se = es.enter_context(
                tc.tile_pool(name="psum_se", bufs=1, space="PSUM"))

            # ---- one-time init (outside the bench loop) ----
            scr = consts.tile([1, 2], F32, name="scr")
            # pin the activation table set (sigmoid_and_others has every
            # function this kernel uses) before any other ACT work
            nc.vector.memset(scr[:], 0.0)
            nc.scalar.activation(scr[:, 0:1], scr[:, 0:1], AF.Sigmoid,
                                 bias=0.0, scale=1.0)
            # padded binary input, pair dim = c-tile (for DoubleRow);
            # b outermost so the two images' byte ranges are disjoint
            zpad = big.tile([128, BP, 2, HP, WPP], FP8, name="zpad", tag="zpad")
            # zpad borders: ct0 half (+-1 encoding) pads with 0.0, ct1 half
            # ({0,1} encoding) pads with 0.5 (z = 2b-1 = 0); borders are
            # never overwritten, so this is loop-invariant
            for bb in range(BP):
                for ct, pad in ((0, 0.0), (1, 0.5)):
                    zp = zpad[:, bb, ct]
                    nc.vector.memset(zp[:, 0, :], pad)
                    nc.vector.memset(zp[:, HP - 1, :], pad)
                    nc.vector.memset(zp[:, :, 0], pad)
                    nc.vector.memset(zp[:, :, 57:], pad)
            tick_sb = None
            if bench_mode:
                tick_sb = consts.tile([1, 8], F32, name="tick_sb")
                nc.vector.memset(tick_sb[:], 1.0)

            if loop_R is not None:
                es.enter_context(tc.For_i(0, loop_R, 1))

            # ---- input stream + constants ----
            # SP queue order = DMA engine order: vecs, b0 x chunks, conv
            # weights, b1 x chunks, tick; everything else issues from the
            # ACT sequencer later so nothing queues ahead of the x stream.
            vecs_all = consts.tile([128, CT, NV], F32, name="vecs_all")
            nc.sync.dma_start(
                vecs_all[:].rearrange("p ct v -> p (ct v)"),
                vecs_d.rearrange("p ct v -> p (ct v)"))
            vecs = [vecs_all[:, ct] for ct in range(CT)]

            wconv_all = consts.tile([128, 9, CT, 2, 128], FP8, name="wconv_all")
            wconv = [[wconv_all[:, t, m] for m in range(CT)] for t in range(9)]

            xts = {}
            for b in range(BP):
                for s in range(NSC2):
                    for ct in range(CT):
                        xt = stream.tile([128, SR2 * W], BF16,
                                         name=f"xt{b}{s}{ct}", tag="xs", bufs=8)
                        nc.sync.dma_start(
                            xt[:],
                            x_v[ct][:, b, s * SR2 * W:(s + 1) * SR2 * W])
                        xts[(b, s, ct)] = xt
                if b == 0:
                    nc.sync.dma_start(
                        wconv_all[:].rearrange("p t m i o -> p (t m i o)"),
                        wa.rearrange("p t m i o -> p (t m i o)"))
            if bench_mode:
                nc.sync.dma_start(tick_d, tick_sb[:])

            w1a_all = consts.tile([128, CT, 2, 32], F32, name="w1a_all")
            w1a = [[w1a_all[:, ct, k] for ct in range(CT)] for k in range(2)]
            w2a = consts.tile([32, C], F32)
            wffn_all = consts.tile([128, CT, CT, 128], BF16, name="wffn_all")
            w1f_all = consts.tile([128, CT, 2, 32], F32, name="w1f_all")
            w1f = [[w1f_all[:, ct, k] for ct in range(CT)] for k in range(2)]
            w2f = consts.tile([32, C], F32)
            b1 = consts.tile([32, 2], F32)
            b1a, b1f = b1[:, 0:1], b1[:, 1:2]

            def load_consts():
                nc.scalar.dma_start(
                    w1a_all[:].rearrange("p ct k o -> p (ct k o)"),
                    w1a_d.rearrange("p ct k o -> p (ct k o)"))
                nc.scalar.dma_start(w2a[:], w2a_d)
                nc.scalar.dma_start(
                    wffn_all[:].rearrange("p a m o -> p (a m o)"),
                    wf.rearrange("p a m o -> p (a m o)"))
                nc.scalar.dma_start(
                    w1f_all[:].rearrange("p ct k o -> p (ct k o)"),
                    w1f_d.rearrange("p ct k o -> p (ct k o)"))
                nc.scalar.dma_start(w2f[:], w2f_d)
                nc.scalar.dma_start(b1[:], b1_d)

            # ---- persistent buffers ----
            # ffn binary input (bf16), kt dim second, b outermost
            z2 = big.tile([128, BP, 2, HW], BF16, name="z2", tag="z2")
            # y1 / u1 share per-(ct,b) slots, bf16
            y1 = [[big.tile([128, HW], BF16, name=f"y1_{ct}_{b}",
                            tag=f"s1_{ct}_{b}")
                   for b in range(BP)] for ct in range(CT)]
            outa = [big.tile([128, PIX], BF16, name=f"outa{ct}", tag=f"oa{ct}")
                    for ct in range(CT)]

            sums2 = [consts.tile([128, 16], F32, name=f"sums2_{ct}")
                     for ct in range(CT)]
            # cols: 8:10 gateA [b], 10:12 gateF [b]
            sxr = [consts.tile([128, 4], F32, name=f"sxr{ct}") for ct in range(CT)]

            ps_y1 = [consts.tile([128, 16], F32, name=f"ps_y1_{ct}")
                     for ct in range(CT)]
            ps_u = [consts.tile([128, 16], F32, name=f"ps_u_{ct}")
                    for ct in range(CT)]
            ps_x = [consts.tile([128, 8], F32, name=f"ps_x_{ct}")
                    for ct in range(CT)]
            ps_oa = [consts.tile([128, 8], F32, name=f"ps_oa_{ct}")
                     for ct in range(CT)]

            # ===== helpers =====
            def phase_a1(b):
                # z (binary sign) first so conv can start ASAP: ct0 via ACT
                # Sign (+-1), ct1 via DVE is_ge ({0,1})
                for s in range(NSC2):
                    for ct in range(CT):
                        xt = xts[(b, s, ct)]
                        zdst = zpad[:, b, ct, 1 + s * SR2:1 + (s + 1) * SR2,
                                    1:1 + W]
                        zsrc = xt[:].rearrange("p (r w) -> p r w", w=W)
                        if ct == 0:
                            nc.scalar.activation(
                                zdst, zsrc, AF.Sign,
                                bias=vecs[ct][:, V_MOVE:V_MOVE + 1], scale=1.0)
                        else:
                            nc.vector.tensor_scalar(
                                zdst, zsrc, vecs[ct][:, V_NMOVE:V_NMOVE + 1],
                                None, op0=ALU.is_ge)
                for s in range(NSC2):
                    for ct in range(CT):
                        xt = xts[(b, s, ct)]
                        seg = slice(b * HW + s * SR2 * W, b * HW + (s + 1) * SR2 * W)
                        if s == 0:
                            nc.scalar.activation(
                                outa[ct][:, seg], xt[:], AF.Prelu, bias=0.0,
                                scale=1.0, alpha=vecs[ct][:, V_AL1:V_AL1 + 1])
                        else:
                            # prelu(x) = max(x, alpha*x) for alpha <= 1 (DVE)
                            nc.vector.scalar_tensor_tensor(
                                outa[ct][:, seg], xt[:],
                                vecs[ct][:, V_AL1:V_AL1 + 1], xt[:],
                                op0=ALU.mult, op1=ALU.max)
                        col = b * NSC2 + s
                        nc.vector.tensor_reduce(
                            ps_x[ct][:, col:col + 1], xt[:],
                            axis=mybir.AxisListType.XY, op=ALU.add)

            def phase_conv(b):
                NW = RS * W
                for mt in range(CT):
                    for jg, spec in ((range(0, 4), [(0, 1), (2, 3)]),
                                     (range(4, 7), [(4, 5), (6,)])):
                        tiles = {}
                        drains = []
                        for js in spec:
                            td = psum_c.tile([128, 2, 512], F32, tag="convd",
                                             bufs=2, name=f"ptd{js[0]}")
                            for q, j in enumerate(js):
                                tiles[j] = td[:, q, 0:NW]
                            drains.append((js, td[:, 0:len(js), 0:NW]))
                        for t in range(9):
                            dy, dx = t // 3, t % 3
                            for j in jg:
                                rhs = zpad[:, b, :,
                                           j * RS + dy:j * RS + dy + RS,
                                           dx:dx + W]
                                nc.tensor.matmul(
                                    tiles[j], wconv[t][mt], rhs,
                                    start=(t == 0), stop=(t == 8), perf_mode=DR)
                        for js, src_ap in drains:
                            j0 = js[0]
                            col = b * 4 + j0 // 2
                            ydst = y1[mt][b][:, j0 * NW:(j0 + len(js)) * NW]
                            nc.scalar.activation(
                                ydst.rearrange("p (q n) -> p q n", n=NW),
                                src_ap,
                                AF.Prelu,
                                bias=vecs[mt][:, V_BIAS1:V_BIAS1 + 1],
                                scale=vecs[mt][:, V_INV1:V_INV1 + 1],
                                alpha=vecs[mt][:, V_AL2:V_AL2 + 1],
                                accum_out=ps_y1[mt][:, col:col + 1])

            def se_gate(b, ps1, ps2, n1, n2, w1k, w2, b1t, vb2, gcol, xcol):
                """SE gate for image b: gate[gcol+b] = sigmoid(...)"""
                for ct in range(CT):
                    nc.vector.tensor_reduce(
                        sxr[ct][:, xcol:xcol + 1],
                        ps1[ct][:, b * n1:(b + 1) * n1],
                        axis=mybir.AxisListType.X, op=ALU.add)
                    nc.vector.tensor_reduce(
                        sxr[ct][:, xcol + 1:xcol + 2],
                        ps2[ct][:, b * n2:(b + 1) * n2],
                        axis=mybir.AxisListType.X, op=ALU.add)
                hp = psum_se.tile([128, 1], F32, tag="seh")
                first = True
                for k in range(2):
                    for ct in range(CT):
                        nc.tensor.matmul(hp[0:32, :], w1k[k][ct],
                                         sxr[ct][:, xcol + k:xcol + k + 1],
                                         start=first,
                                         stop=(k == 1 and ct == CT - 1))
                        first = False
                hs = consts.tile([32, 1], F32, tag="hs")
                nc.scalar.activation(hs[:], hp[0:32, :], AF.Relu, bias=b1t,
                                     scale=1.0)
                for mt in range(CT):
                    gp = psum_se.tile([128, 1], F32, tag="seh", name="gp")
                    nc.tensor.matmul(gp[:], w2[:, mt * 128:(mt + 1) * 128], hs[:],
                                     start=True, stop=True)
                    nc.scalar.activation(
                        sums2[mt][:, gcol + b:gcol + b + 1], gp[:], AF.Sigmoid,
                        bias=vecs[mt][:, vb2:vb2 + 1], scale=1.0)

            def phase_a4(b):
                for ct in range(CT):
                    for s in range(NSC2):
                        seg = slice(b * HW + s * SR2 * W,
                                    b * HW + (s + 1) * SR2 * W)
                        yseg = slice(s * SR2 * W, (s + 1) * SR2 * W)
                        tmp = stream.tile([128, SR2 * W], BF16, tag="work", bufs=6)
                        nc.vector.tensor_tensor(
                            tmp[:], y1[ct][b][:, yseg], outa[ct][:, seg],
                            op=ALU.mult)
                        col = b * NSC2 + s
                        nc.vector.scalar_tensor_tensor(
                            outa[ct][:, seg], tmp[:],
                            sums2[ct][:, 8 + b:9 + b], outa[ct][:, seg],
                            op0=ALU.mult, op1=ALU.add,
                            accum_out=ps_oa[ct][:, col:col + 1])
                        # ffn binary input: ct0 +-1 via ACT Sign, ct1 {0,1}
                        # via DVE is_ge (bf16 out -> 2x mode)
                        z2dst = z2[:, b, ct, s * SR2 * W:(s + 1) * SR2 * W]
                        if ct == 0:
                            nc.scalar.activation(
                                z2dst, outa[ct][:, seg], AF.Sign,
                                bias=vecs[ct][:, V_ZB2:V_ZB2 + 1], scale=1.0)
                        else:
                            nc.vector.tensor_scalar(
                                z2dst, outa[ct][:, seg],
                                vecs[ct][:, V_NZB2:V_NZB2 + 1], None,
                                op0=ALU.is_ge)

            def phase_f1(b, dve_drains=False):
                for mt in range(CT):
                    for j in range(NCH):
                        pt = psum_f.tile([128, RS * W], F32, tag="ffn", bufs=2)
                        for kt in range(CT):
                            nc.tensor.matmul(
                                pt[:], wffn_all[:, kt, mt],
                                z2[:, b, kt, j * RS * W:(j + 1) * RS * W],
                                start=(kt == 0), stop=(kt == CT - 1))
                        col = b * NCH + j
                        udst = y1[mt][b][:, j * RS * W:(j + 1) * RS * W]
                        if dve_drains and j % 2 == 1:
                            # 2-op DVE drain: t = p*scale + bias, then
                            # u1 = max(t, alpha*t); frees the ACT queue so
                            # the tail image's SE sums complete sooner
                            td = stream.tile([128, RS * W], BF16, tag="work",
                                             bufs=6, name="td")
                            nc.vector.tensor_scalar(
                                td[:], pt[:],
                                vecs[mt][:, V_FINV1:V_FINV1 + 1],
                                vecs[mt][:, V_FBIAS1:V_FBIAS1 + 1],
                                op0=ALU.mult, op1=ALU.add)
                            nc.vector.scalar_tensor_tensor(
                                udst, td[:],
                                vecs[mt][:, V_FAL2:V_FAL2 + 1], td[:],
                                op0=ALU.mult, op1=ALU.max,
                                accum_out=ps_u[mt][:, col:col + 1])
                        else:
                            nc.scalar.activation(
                                udst, pt[:],
                                AF.Prelu,
                                bias=vecs[mt][:, V_FBIAS1:V_FBIAS1 + 1],
                                scale=vecs[mt][:, V_FINV1:V_FINV1 + 1],
                                alpha=vecs[mt][:, V_FAL2:V_FAL2 + 1],
                                accum_out=ps_u[mt][:, col:col + 1])

            def phase_f3(b):
                for ct in range(CT):
                    for s in range(NSC2):
                        seg = slice(b * HW + s * SR2 * W,
                                    b * HW + (s + 1) * SR2 * W)
                        yseg = slice(s * SR2 * W, (s + 1) * SR2 * W)
                        t2 = stream.tile([128, SR2 * W], BF16, tag="work", bufs=6)
                        nc.vector.tensor_scalar(
                            t2[:], y1[ct][b][:, yseg],
                            sums2[ct][:, 10 + b:11 + b],
                            vecs[ct][:, V_CFIN:V_CFIN + 1],
                            op0=ALU.mult, op1=ALU.add)
                        fin = stream.tile([128, SR2 * W], BF16, tag="work",
                                          bufs=6, name="fin")
                        nc.vector.tensor_tensor(
                            fin[:], t2[:], outa[ct][:, seg], op=ALU.add)
                        nc.sync.dma_start(
                            out_v[ct][:, b, s * SR2 * W:(s + 1) * SR2 * W],
                            fin[:])

            # ===== schedule =====
            # b0's gate + postprocessing are emitted BETWEEN conv(0) and
            # conv(1) so they sit ahead of conv-b1 in the per-engine FIFOs
            # and execute during conv-b1's matmuls.
            if phase_limit >= 1:
                phase_a1(0)
            load_consts()
            if phase_limit >= 1:
                phase_a1(1)
            if phase_limit >= 2:
                phase_conv(0)
            if phase_limit >= 3:
                se_gate(0, ps_x, ps_y1, NSC2, NCH, w1a, w2a, b1a, V_B2A, 8, 0)
            if phase_limit >= 4:
                phase_a4(0)
            if phase_limit >= 2:
                phase_conv(1)
            if phase_limit >= 3:
                se_gate(1, ps_x, ps_y1, NSC2, NCH, w1a, w2a, b1a, V_B2A, 8, 0)
            if phase_limit >= 5:
                phase_f1(0)
            if phase_limit >= 4:
                phase_a4(1)
            if phase_limit >= 6:
                se_gate(0, ps_oa, ps_u, NSC2, NCH, w1f, w2f, b1f, V_B2F, 10, 2)
            if phase_limit >= 7:
                phase_f3(0)
            if phase_limit >= 5:
                phase_f1(1, dve_drains=True)
            if phase_limit >= 6:
                se_gate(1, ps_oa, ps_u, NSC2, NCH, w1f, w2f, b1f, V_B2F, 10, 2)
            if phase_limit >= 7:
                phase_f3(1)

    nc.compile()
    return nc


def _host_prep(inputs):
    import ml_dtypes
    f32 = np.float32
    fp8 = ml_dtypes.float8_e4m3
    bf16 = ml_dtypes.bfloat16

    g1, be1, m1, v1 = (inputs["a_bn1"][i].astype(f32) for i in range(4))
    g2, be2, m2, v2 = (inputs["a_bn2"][i].astype(f32) for i in range(4))
    fg1, fbe1, fm1, fv1 = (inputs["f_bn1"][i].astype(f32) for i in range(4))
    fg2, fbe2, fm2, fv2 = (inputs["f_bn2"][i].astype(f32) for i in range(4))
    inv1 = g1 / np.sqrt(v1 + EPS)
    bias1 = be1 - m1 * inv1
    inv2 = g2 / np.sqrt(v2 + EPS)
    bias2 = be2 - m2 * inv2
    finv1 = fg1 / np.sqrt(fv1 + EPS)
    fbias1 = fbe1 - fm1 * finv1
    finv2 = fg2 / np.sqrt(fv2 + EPS)
    fbias2 = fbe2 - fm2 * finv2

    s = inputs["a_scale"].astype(f32)
    fs = inputs["f_scale"].astype(f32)

    # conv weights: ct0 half (+-1 z): W512 = fp8(512 w); ct1 half ({0,1} b):
    # W1024 = fp8(1024 w), z = 2b-1 with border b=0.5:
    #   conv = psum/512 - S1,  S1[m] = sum_{c in ct1, tap} W1024[m,c]/1024
    bw = np.clip(inputs["a_w"].astype(f32), -1.0, 1.0)
    awq = np.empty((C, C, 3, 3), fp8)
    awq[:, :128] = (bw[:, :128] * 512.0).astype(fp8)
    awq[:, 128:] = (bw[:, 128:] * 1024.0).astype(fp8)
    S1 = awq[:, 128:].astype(f32).sum(axis=(1, 2, 3)) / 1024.0   # [O]

    # ffn weights (bf16): ct0 half: w; ct1 half: 2w -> u = psum - S1f
    bwf = np.clip(inputs["f_w"].astype(f32), -1.0, 1.0)
    fwq = np.empty((C, C), bf16)
    fwq[:, :128] = bwf[:, :128].astype(bf16)
    fwq[:, 128:] = (bwf[:, 128:] * 2.0).astype(bf16)
    S1f = fwq[:, 128:].astype(f32).sum(axis=1) / 2.0             # [O]

    # wa[p, t, mt, i, m] = awq[mt*128+m, i*128+p, ky, kx]
    aw4 = awq.reshape(CT, 128, CT, 128, 3, 3)             # [mt, m, i, p, ky, kx]
    wa_h = np.ascontiguousarray(
        np.transpose(aw4, (3, 4, 5, 0, 2, 1)).reshape(128, 9, CT, 2, 128))
    # wf[p, kt, mt, m] = fwq[mt*128+m, kt*128+p]
    fw4 = fwq.reshape(CT, 128, CT, 128)                   # [mt, m, kt, p]
    wf_h = np.ascontiguousarray(
        np.transpose(fw4, (3, 2, 0, 1)).reshape(128, CT, CT, 128))

    vecs = np.zeros((C, NV), f32)
    vecs[:, 0] = inputs["a_move"].astype(f32)             # V_MOVE (ACT bias)
    vecs[:, 1] = inputs["a_alpha1"]
    # inv2 folded into the conv drain (prelu is positive-homogeneous)
    vecs[:, 2] = inv1 * inv2 / 512.0                      # V_INV1
    vecs[:, 3] = inv2 * (bias1 - inv1 * S1)               # V_BIAS1
    vecs[:, 4] = inputs["a_alpha2"]
    vecs[:, 5] = -inputs["a_move"].astype(f32)            # V_NMOVE (DVE thr)
    vecs[:, 7] = bias2 + inputs["f_move"].astype(f32)     # V_ZB2 (ACT bias)
    vecs[:, 8] = -(bias2 + inputs["f_move"].astype(f32))  # V_NZB2 (DVE thr)
    vecs[:, 9] = finv1 * finv2                            # V_FINV1
    vecs[:, 10] = finv2 * (fbias1 - finv1 * S1f)          # V_FBIAS1
    vecs[:, 11] = inputs["f_alpha2"]
    vecs[:, 12] = fs
    vecs[:, 13] = 1.0 - fs
    vecs[:, 14] = finv2
    vecs[:, 15] = fbias2 + bias2                          # V_CFIN
    vecs[:, 16] = inputs["a_se_b2"]
    vecs[:, 17] = inputs["f_se_b2"]
    vecs_ct = np.ascontiguousarray(
        np.transpose(vecs.reshape(CT, 128, NV), (1, 0, 2)))  # [128, CT, NV]

    # SE first-layer weights: fold s/(1-s), 1/HW; the y1/u1 sums arrive
    # pre-scaled by inv2/finv2, so divide that back out of the k=1 column.
    def w1_fold(w1, sv, invk):
        w1t = w1.astype(f32).T / float(HW)          # [256, 32]
        out = np.zeros((CT, 2, 128, 32), f32)
        out[:, 0] = (w1t * sv[:, None]).reshape(CT, 128, 32)
        out[:, 1] = (w1t * ((1.0 - sv) / invk)[:, None]).reshape(CT, 128, 32)
        return np.ascontiguousarray(np.transpose(out, (2, 0, 1, 3)))

    w1a_h = w1_fold(inputs["a_se_w1"], s, inv2)
    w1f_h = w1_fold(inputs["f_se_w1"], fs, finv2)

    # f-SE pools mean(mix2) where mix2 uses y_att = outa + bias2; the outa
    # sums miss HW*bias2 per channel -> fold into the SE hidden bias.
    b1f_adj = inputs["f_se_b1"].astype(f32) + (
        np.transpose(w1f_h[:, :, 0, :], (1, 0, 2)).reshape(C, 32)
        * (float(HW) * bias2)[:, None]
    ).sum(axis=0)
    b1 = np.stack([inputs["a_se_b1"].astype(f32), b1f_adj], axis=1)

    common = {
        "wa": wa_h, "wf": wf_h, "vecs": vecs_ct, "b1": np.ascontiguousarray(b1),
        "w1a": w1a_h,
        "w2a": np.ascontiguousarray(inputs["a_se_w2"].astype(f32).T),
        "w1f": w1f_h,
        "w2f": np.ascontiguousarray(inputs["f_se_w2"].astype(f32).T),
    }
    return common


def kernel(**inputs):
    import ml_dtypes
    from concourse import bass_utils

    if "nc" not in _CACHE:
        _CACHE["nc"] = _build_program()
    nc = _CACHE["nc"]

    common = _host_prep(inputs)
    x16 = inputs["x"].astype(ml_dtypes.bfloat16)
    in_maps = []
    for c in range(NCORES):
        m = dict(common)
        m["xin"] = np.ascontiguousarray(x16[c * BP:(c + 1) * BP])
        in_maps.append(m)

    res = None
    for attempt in range(3):
        try:
            res = bass_utils.run_bass_kernel_spmd(
                nc, in_maps, core_ids=list(range(NCORES)))
            break
        except Exception:
            # transient device wedge on a freshly loaded NEFF: retry
            if attempt == 2:
                raise
    out = np.empty((B, C, H, W), np.float32)
    for c in range(NCORES):
        out[c * BP:(c + 1) * BP] = res.results[c]["out"].astype(np.float32)
    return out
